# revision 6
# baseline (speedup 1.0000x reference)
"""CRF Viterbi decode (B=1024, T=1024, N=32) on 8 TRN2 NeuronCores — v2.

Data-parallel: batch split 128/core, [32,32] transition replicated.

vs v1: (a) snapshot removed from the forward serial chain (batched
masked-sum gather after the forward); (b) forward add+reduce split
across DVE and GPSIMD by cur-segment so each engine chains its own
add->reduce in program order; (c) backtrace freeze handled by a one-time
rewrite states[t] := BIG*onehot(last_tag) for t >= len-1, so the
per-step chain is just onehot -> transpose -> 4x PE matmul -> custom
argmax writing tags[:, t] directly.  All f32 ops identical in value to
the reference (max/adds in same positions), so output stays exact.
"""
import sys
sys.path.insert(0, "/opt/trn_rl_repo")

import numpy as np

import concourse.bass as bass
import concourse.bacc as bacc
import concourse.mybir as mybir
import concourse.tile as tile
from concourse.bass_utils import run_bass_kernel_spmd

F32 = mybir.dt.float32
I32 = mybir.dt.int32
I8 = mybir.dt.int8
AX = mybir.AxisListType
OP = mybir.AluOpType

B, T, N = 1024, 1024, 32
PB = 128
NCORES = 8
BIG = 1.0e6

_ops_cache = {}
_nc_cache = {}
_last_exec_ns = [None]


def register_custom_ops():
    if _ops_cache:
        return _ops_cache["BT32"]
    from concourse.dve_spec import (
        Spec, Src0, Src1, AluOp, lower, Idx, scan, Scan, MaxNeg,
    )
    from concourse.dve_ops import DveOp, OPS, has_src1
    from concourse.dve_uop import DveOpSpec, AluInp
    import concourse.dve_ops as dom

    def make(name, spec, subdim, patch=None):
        for o in OPS:
            if o.name == name:
                return o
        OPS_len = len(OPS)
        dom._SUB_OPCODE_FOR_NAME[name] = dom._CUSTOM_DVE_ROW_BASE + OPS_len
        assert dom._SUB_OPCODE_FOR_NAME[name] < 0x20
        shas = {}
        for ver in ("v3", "v4"):
            uops = lower(spec, ver=ver)
            if patch is not None:
                patch(uops)
            s = DveOpSpec(name=name, opcode=dom.get_dve_sub_opcode(name),
                          uops=uops, rd1_en=has_src1(spec))
            shas[ver] = s.sha(ver)
            dom._COMPILE_CACHE[(name, ver)] = s
        op = DveOp(name, spec, subdim=subdim, uops_sha=shas)
        OPS.append(op)
        dom.CUSTOM_DVE_SPECS[name] = spec
        return op

    def make_segmax():
        """Segmented running-max of (Src0 + Src1) with reset at each subdim
        (row) boundary.  lower() gives the PageIdx FSM skeleton
        [seed, steady(hold), step(combine)]; patch the scan stage so
        steady combines max(reg, incoming) and step resets reg to the
        incoming element (first element of the new segment)."""
        FMAX = np.float32(3.4028235e38)

        def ref(in0, in1, c0, c1, c2):
            x = (in0 + in1).astype(np.float32)
            r = np.maximum.accumulate(x, axis=-1)
            return r, None

        sc = Scan(AluOp.MAX, Src0 + Src1, _subdim_step=MaxNeg)
        spec = Spec(body=sc, reference=ref)

        def patch(uops):
            assert len(uops) == 3, f"expected [seed, steady, step], got {len(uops)}"
            steady, step = uops[1], uops[2]
            dp = steady.datapath_config[1]
            dp.op = AluOp.MAX
            dp.alu_src0 = AluInp.CURR_ALU_OUT
            dp.alu_src1 = AluInp.PREV_ALU_OUT
            dp = step.datapath_config[1]
            dp.op = AluOp.BYPASS
            dp.alu_src0 = AluInp.PREV_ALU_OUT
            dp.alu_src1 = AluInp.PREV_ALU_OUT

        return make("CRF_SEGMAX", spec, subdim=True, patch=patch)

    FMAX = np.float32(3.4028235e38)

    def ref3(in0, in1, c0, c1, c2):
        P, K = in0.shape
        x = (in0 + in1).astype(np.float32)
        r = np.maximum.accumulate(x, axis=1)
        m = ((x == r).astype(np.float32) * np.arange(K, dtype=np.float32)[None, :])
        return m, m.max(axis=1, initial=-FMAX).reshape(P, 1)

    from concourse.dve_spec import eq
    _x3 = Src0 + Src1
    spec3 = Spec(body=eq(_x3, scan(AluOp.MAX, _x3)) * Idx, accum=AluOp.MAX,
                 reference=ref3)

    op3 = make("CRF_BT32", spec3, subdim=False)

    # BT32X: same body/accum, but the per-element OUTPUT is redirected to
    # the accumulator chain (block-7 ALU_OUT), so out[k] = running
    # max(eq(x, runmax(x)) * Idx); out[31] = the encoded argmax.  This
    # removes the separate DVE_READ_ACCUMULATOR2 instruction per step.
    from concourse.dve_uop import OutPath, OutSel

    def ref3x(in0, in1, c0, c1, c2):
        P, K = in0.shape
        x = (in0 + in1).astype(np.float32)
        r = np.maximum.accumulate(x, axis=1)
        m = ((x == r).astype(np.float32) * np.arange(K, dtype=np.float32)[None, :])
        acc = np.maximum.accumulate(
            np.maximum(m, -FMAX), axis=1).astype(np.float32)
        return acc, acc[:, -1:].copy()

    spec3x = Spec(body=eq(_x3, scan(AluOp.MAX, _x3)) * Idx, accum=AluOp.MAX,
                  reference=ref3x)

    def patch_out(uops):
        steady = uops[-1]
        assert steady.out_enable[OutPath.WR0_LO]
        steady.out[OutPath.WR0_LO] = OutSel.ALU_OUT

    op3x = make("CRF_BT32X", spec3x, subdim=False, patch=patch_out)
    opF = make_segmax()
    _ops_cache["BT32"] = op3
    _ops_cache["BT32X"] = op3x
    _ops_cache["SEGMAX"] = opF
    return op3


def build_nc(Tn, CH=64):
    if Tn < CH:
        CH = Tn
    op3 = register_custom_ops()
    opF = _ops_cache["SEGMAX"]

    nc = bacc.Bacc("TRN2", target_bir_lowering=False, debug=False,
                   num_devices=NCORES)

    logits = nc.dram_tensor("logits", [PB, Tn, N], F32, kind="ExternalInput")
    trep_d = nc.dram_tensor("trep", [PB, N * N], F32, kind="ExternalInput")
    meq_d = nc.dram_tensor("meq", [PB, Tn], F32, kind="ExternalInput")
    mlt_d = nc.dram_tensor("mlt", [PB, Tn], F32, kind="ExternalInput")
    mrw_d = nc.dram_tensor("mrw", [PB, Tn], I8, kind="ExternalInput")
    mrwf_d = nc.dram_tensor("mrwf", [PB, Tn], F32, kind="ExternalInput")
    mrwc_d = nc.dram_tensor("mrwc", [PB, Tn], F32, kind="ExternalInput")
    irev_d = nc.dram_tensor("irev", [PB, N], F32, kind="ExternalInput")
    trevw_d = nc.dram_tensor("trevw", [PB, N], F32, kind="ExternalInput")
    out_d = nc.dram_tensor("out", [PB, Tn], I32, kind="ExternalOutput")

    SCH = 64  # snapshot-gather chunk (time steps per chunk)
    nsch = (Tn + SCH - 1) // SCH

    with tile.TileContext(nc) as tc:
        with (
            tc.tile_pool(name="consts", bufs=1) as cpool,
            tc.tile_pool(name="states", bufs=1) as spool,
            tc.tile_pool(name="big", bufs=1) as bpool,
            tc.tile_pool(name="lchunks", bufs=2) as lpool,
            tc.tile_pool(name="small", bufs=1) as mpool,
            tc.tile_pool(name="psum", bufs=1, space="PSUM") as ppool,
        ):
            trep = cpool.tile([PB, N * N], F32, tag="trep")
            meq = cpool.tile([PB, Tn], F32, tag="meq")
            mlt = cpool.tile([PB, Tn], F32, tag="mlt")
            mrw = cpool.tile([PB, Tn], I8, tag="mrw")
            mrwf = cpool.tile([PB, Tn], F32, tag="mrwf")
            mrwc = cpool.tile([PB, Tn], F32, tag="mrwc")
            rsc = bpool.tile([PB, 64 * N], F32, tag="rsc")
            irev = cpool.tile([PB, N], F32, tag="irev")
            trevw = cpool.tile([PB, N], F32, tag="trevw")
            states = spool.tile([PB, Tn * N], F32, tag="states")
            scores = bpool.tile([PB, N * N], F32, tag="scores")
            tags = bpool.tile([PB, Tn], F32, tag="tags")
            outi = bpool.tile([PB, Tn], I32, tag="outi")
            scratch = bpool.tile([PB, SCH * N], F32, tag="scratch")
            snap = mpool.tile([PB, N], F32, tag="snap")
            lastt = mpool.tile([PB, 1], F32, tag="lastt")
            snapc = mpool.tile([PB, N], F32, tag="snapc")
            eqs = mpool.tile([PB, N], F32, tag="eqs")
            red = mpool.tile([PB, N], F32, tag="red")
            m1 = mpool.tile([PB, 1], F32, tag="m1")
            onehot = mpool.tile([PB, N], F32, tag="onehot")
            bigoh = mpool.tile([PB, N], F32, tag="bigoh")
            onehotT = mpool.tile([PB, N], F32, tag="onehotT")
            tsel = ppool.tile([PB, N], F32, tag="tsel")

            # first logits chunk + trep gate the first forward step: issue
            # their DMAs before the other constants
            lt0 = lpool.tile([PB, CH * N], F32, tag="lchunk")
            nc.sync.dma_start(
                out=lt0[:].rearrange("p (t v) -> p t v", v=N),
                in_=logits.ap()[:, 0:CH, :],
            )
            nc.sync.dma_start(out=trep[:], in_=trep_d.ap())
            nc.sync.dma_start(out=meq[:], in_=meq_d.ap())
            nc.sync.dma_start(out=mlt[:], in_=mlt_d.ap())
            nc.sync.dma_start(out=mrw[:], in_=mrw_d.ap())
            nc.sync.dma_start(out=mrwf[:], in_=mrwf_d.ap())
            nc.sync.dma_start(out=mrwc[:], in_=mrwc_d.ap())
            nc.sync.dma_start(out=irev[:], in_=irev_d.ap())
            nc.sync.dma_start(out=trevw[:], in_=trevw_d.ap())

            trep3 = trep[:].rearrange("p (c v) -> p c v", v=N)
            scores3 = scores[:].rearrange("p (c v) -> p c v", v=N)

            # ---------------- forward ----------------
            nchunks = (Tn + CH - 1) // CH
            for c in range(nchunks):
                if c == 0:
                    lt = lt0
                else:
                    lt = lpool.tile([PB, CH * N], F32, tag="lchunk")
                    nc.sync.dma_start(
                        out=lt[:].rearrange("p (t v) -> p t v", v=N),
                        in_=logits.ap()[:, c * CH:(c + 1) * CH, :],
                    )
                if c == 0:
                    nc.vector.tensor_copy(out=states[:, 0:N], in_=lt[:, 0:N])
                for i in range(CH):
                    t = c * CH + i
                    if t == 0:
                        continue
                    sprev = states[:, (t - 1) * N: t * N]
                    scur = states[:, t * N: (t + 1) * N]
                    sprev_b = sprev.rearrange("p (o v) -> p o v", o=1) \
                                   .to_broadcast((PB, N, N))
                    # fused add + segmented running-max; segment max lands
                    # in the last column of each row of scores3
                    nc.vector._custom_dve(
                        opF, out=scores3, in0=sprev_b, in1=trep3)
                    nc.vector.tensor_tensor(
                        out=scur.rearrange("p (c o) -> p c o", o=1),
                        in0=scores3[:, :, N - 1:N],
                        in1=lt[:, i * N:(i + 1) * N]
                            .rearrange("p (c o) -> p c o", o=1),
                        op=OP.add)

            # ------------- snapshot gather (masked sum over t) -------------
            # All accumulation on the idle Pool engine (exact: every term
            # but states[len-1] is +/-0.0, and x + 0.0 == x in f32); one
            # final strided reduce on DVE.
            states3 = states[:].rearrange("p (t v) -> p t v", v=N)
            scratch3 = scratch[:].rearrange("p (t v) -> p t v", v=N)
            rsc3s = rsc[:].rearrange("p (t v) -> p t v", v=N)
            nc.gpsimd.memset(scratch[:], 0.0)
            for k in range(nsch):
                t0 = k * SCH
                meqb = meq[:, t0:t0 + SCH] \
                    .rearrange("p (t o) -> p t o", o=1).to_broadcast((PB, SCH, N))
                nc.gpsimd.tensor_tensor(
                    out=rsc3s, in0=states3[:, t0:t0 + SCH, :], in1=meqb,
                    op=OP.mult)
                nc.gpsimd.tensor_tensor(
                    out=scratch3, in0=scratch3, in1=rsc3s, op=OP.add)
            # reduce over t: view scratch as [p, v, t] (strided inner t)
            sc_vt = scratch[:].rearrange("p (t v) -> p v t", v=N)
            nc.vector.tensor_reduce(
                out=snap[:], in_=sc_vt, axis=AX.X, op=OP.add)

            # ------------- last tag (exact first-index argmax) -------------
            nc.vector.tensor_reduce(out=m1[:], in_=snap[:], axis=AX.X, op=OP.max)
            nc.vector.tensor_scalar(
                out=eqs[:], in0=snap[:], scalar1=m1[:], scalar2=None,
                op0=OP.is_equal)
            nc.vector.tensor_tensor(out=red[:], in0=eqs[:], in1=irev[:],
                                    op=OP.mult)
            nc.vector.tensor_reduce(out=lastt[:], in_=red[:],
                                    axis=AX.X, op=OP.max)

            # ------------- rewrite states[t >= len-1] = BIG*onehot ---------
            # Top rows (needed first by the backtrace) via DVE
            # copy_predicated; lower rows rewritten on the idle Pool engine
            # while the backtrace descends (states = states*(1-m) + bigoh*m).
            RS = 896
            RCH = 64
            nc.vector.tensor_scalar(
                out=onehot[:], in0=irev[:], scalar1=lastt[:],
                scalar2=None, op0=OP.is_equal)
            nc.vector.tensor_scalar(
                out=bigoh[:], in0=onehot[:], scalar1=BIG, scalar2=None,
                op0=OP.mult)
            mrwb = mrw[:, RS:].rearrange("p (t o) -> p t o", o=1) \
                .to_broadcast((PB, Tn - RS, N))
            bigohb = bigoh[:].rearrange("p (o v) -> p o v", o=1) \
                .to_broadcast((PB, Tn - RS, N))
            nc.vector.copy_predicated(out=states3[:, RS:, :], mask=mrwb,
                                      data=bigohb)
            rsc3 = rsc[:].rearrange("p (t v) -> p t v", v=N)
            bigohc = bigoh[:].rearrange("p (o v) -> p o v", o=1) \
                .to_broadcast((PB, RCH, N))
            for t0 in range(RS - RCH, -1, -RCH):
                stc = states3[:, t0:t0 + RCH, :]
                mc = mrwc[:, t0:t0 + RCH] \
                    .rearrange("p (t o) -> p t o", o=1) \
                    .to_broadcast((PB, RCH, N))
                mf = mrwf[:, t0:t0 + RCH] \
                    .rearrange("p (t o) -> p t o", o=1) \
                    .to_broadcast((PB, RCH, N))
                nc.gpsimd.tensor_tensor(out=stc, in0=stc, in1=mc, op=OP.mult)
                nc.gpsimd.tensor_tensor(out=rsc3, in0=bigohc, in1=mf,
                                        op=OP.mult)
                nc.gpsimd.tensor_tensor(out=stc, in0=stc, in1=rsc3,
                                        op=OP.add)

            # ---------------- backtrace ----------------
            nc.vector.tensor_copy(out=tags[:, Tn - 1:Tn], in_=lastt[:])
            for t in range(Tn - 2, -1, -1):
                st = states[:, t * N: (t + 1) * N]
                nc.vector.tensor_scalar(
                    out=onehot[:], in0=irev[:], scalar1=tags[:, t + 1:t + 2],
                    scalar2=None, op0=OP.is_equal)
                nc.vector.transpose(out=onehotT[:], in_=onehot[:])
                for blk in range(4):
                    nc.tensor.matmul(
                        out=tsel[blk * N:(blk + 1) * N, :],
                        lhsT=onehotT[blk * N:(blk + 1) * N, :],
                        rhs=trevw[blk * N:(blk + 1) * N, :],
                        start=True, stop=True,
                        tile_position=(blk * N, blk * N))
                nc.vector._custom_dve(
                    op3, out=red[:], in0=tsel[:], in1=st[:, ::-1],
                    accum_out=tags[:, t:t + 1])

            # ---------------- decode + mask + output ----------------
            nc.vector.tensor_scalar(
                out=tags[:], in0=tags[:], scalar1=-1.0, scalar2=31.0,
                op0=OP.mult, op1=OP.add)
            nc.vector.tensor_tensor(out=outi[:], in0=tags[:], in1=mlt[:],
                                    op=OP.mult)
            nc.sync.dma_start(out=out_d.ap(), in_=outi[:])

    nc.compile()
    return nc


def make_inputs_for_core(logits_shard, lens_shard, Tn, Tmat):
    trep = np.ascontiguousarray(Tmat.T).reshape(1, N * N)
    tcol = np.arange(Tn)[None, :]
    lens = lens_shard.astype(np.int64)[:, None]
    meq = (lens == (tcol + 1)).astype(np.float32)
    mlt = (tcol < lens).astype(np.float32)
    mrw = (tcol >= (lens - 1)).astype(np.int8)
    irev = (31.0 - np.arange(N, dtype=np.float32))[None, :]
    rep = lambda a: np.ascontiguousarray(
        np.broadcast_to(a, (PB, a.shape[1])), dtype=np.float32)
    return {
        "logits": np.ascontiguousarray(logits_shard, dtype=np.float32),
        "trep": rep(trep),
        "meq": np.ascontiguousarray(meq, dtype=np.float32),
        "mlt": np.ascontiguousarray(mlt, dtype=np.float32),
        "mrw": np.ascontiguousarray(mrw, dtype=np.int8),
        "mrwf": np.ascontiguousarray(mrw, dtype=np.float32),
        "mrwc": np.ascontiguousarray(1 - mrw, dtype=np.float32),
        "irev": rep(irev),
        "trevw": np.ascontiguousarray(
            np.tile(Tmat[::-1, :].T, (4, 1)), dtype=np.float32),
    }


def last_exec_time_ns():
    return _last_exec_ns[0]


def kernel(logits, transitions, sequence_lengths, _trace=False):
    logits = np.asarray(logits, dtype=np.float32)
    Tmat = np.asarray(transitions, dtype=np.float32)
    lens = np.asarray(sequence_lengths)
    Bn, Tn, Nn = logits.shape
    assert Nn == N and Bn % NCORES == 0

    if Tn not in _nc_cache:
        _nc_cache[Tn] = build_nc(Tn)
    nc = _nc_cache[Tn]

    in_maps = []
    for i in range(NCORES):
        sl = slice(i * PB, (i + 1) * PB)
        in_maps.append(make_inputs_for_core(logits[sl], lens[sl], Tn, Tmat))

    kw = {}
    if _trace:
        kw = dict(trace=True, trace_cores=[0])
    res = run_bass_kernel_spmd(nc, in_maps, core_ids=list(range(NCORES)), **kw)
    _last_exec_ns[0] = getattr(res, "exec_time_ns", None)

    out = np.concatenate([res.results[i]["out"] for i in range(NCORES)], axis=0)
    return out.astype(np.int32)


# revision 7
# speedup vs baseline: 1.0006x; 1.0006x over previous
"""CRF Viterbi decode (B=1024, T=1024, N=32) on 8 TRN2 NeuronCores — v2.

Data-parallel: batch split 128/core, [32,32] transition replicated.

vs v1: (a) snapshot removed from the forward serial chain (batched
masked-sum gather after the forward); (b) forward add+reduce split
across DVE and GPSIMD by cur-segment so each engine chains its own
add->reduce in program order; (c) backtrace freeze handled by a one-time
rewrite states[t] := BIG*onehot(last_tag) for t >= len-1, so the
per-step chain is just onehot -> transpose -> 4x PE matmul -> custom
argmax writing tags[:, t] directly.  All f32 ops identical in value to
the reference (max/adds in same positions), so output stays exact.
"""
import sys
sys.path.insert(0, "/opt/trn_rl_repo")

import numpy as np

import concourse.bass as bass
import concourse.bacc as bacc
import concourse.mybir as mybir
import concourse.tile as tile
from concourse.bass_utils import run_bass_kernel_spmd

F32 = mybir.dt.float32
I32 = mybir.dt.int32
I8 = mybir.dt.int8
AX = mybir.AxisListType
OP = mybir.AluOpType

B, T, N = 1024, 1024, 32
PB = 128
NCORES = 8
BIG = 1.0e6

_ops_cache = {}
_nc_cache = {}
_last_exec_ns = [None]


def register_custom_ops():
    if _ops_cache:
        return _ops_cache["BT32"]
    from concourse.dve_spec import (
        Spec, Src0, Src1, AluOp, lower, Idx, scan, Scan, MaxNeg,
    )
    from concourse.dve_ops import DveOp, OPS, has_src1
    from concourse.dve_uop import DveOpSpec, AluInp
    import concourse.dve_ops as dom

    def make(name, spec, subdim, patch=None):
        for o in OPS:
            if o.name == name:
                return o
        OPS_len = len(OPS)
        dom._SUB_OPCODE_FOR_NAME[name] = dom._CUSTOM_DVE_ROW_BASE + OPS_len
        assert dom._SUB_OPCODE_FOR_NAME[name] < 0x20
        shas = {}
        for ver in ("v3", "v4"):
            uops = lower(spec, ver=ver)
            if patch is not None:
                patch(uops)
            s = DveOpSpec(name=name, opcode=dom.get_dve_sub_opcode(name),
                          uops=uops, rd1_en=has_src1(spec))
            shas[ver] = s.sha(ver)
            dom._COMPILE_CACHE[(name, ver)] = s
        op = DveOp(name, spec, subdim=subdim, uops_sha=shas)
        OPS.append(op)
        dom.CUSTOM_DVE_SPECS[name] = spec
        return op

    def make_segmax():
        """Segmented running-max of (Src0 + Src1) with reset at each subdim
        (row) boundary.  lower() gives the PageIdx FSM skeleton
        [seed, steady(hold), step(combine)]; patch the scan stage so
        steady combines max(reg, incoming) and step resets reg to the
        incoming element (first element of the new segment)."""
        FMAX = np.float32(3.4028235e38)

        def ref(in0, in1, c0, c1, c2):
            x = (in0 + in1).astype(np.float32)
            r = np.maximum.accumulate(x, axis=-1)
            return r, None

        sc = Scan(AluOp.MAX, Src0 + Src1, _subdim_step=MaxNeg)
        spec = Spec(body=sc, reference=ref)

        def patch(uops):
            assert len(uops) == 3, f"expected [seed, steady, step], got {len(uops)}"
            steady, step = uops[1], uops[2]
            dp = steady.datapath_config[1]
            dp.op = AluOp.MAX
            dp.alu_src0 = AluInp.CURR_ALU_OUT
            dp.alu_src1 = AluInp.PREV_ALU_OUT
            dp = step.datapath_config[1]
            dp.op = AluOp.BYPASS
            dp.alu_src0 = AluInp.PREV_ALU_OUT
            dp.alu_src1 = AluInp.PREV_ALU_OUT

        return make("CRF_SEGMAX", spec, subdim=True, patch=patch)

    FMAX = np.float32(3.4028235e38)

    def ref3(in0, in1, c0, c1, c2):
        P, K = in0.shape
        x = (in0 + in1).astype(np.float32)
        r = np.maximum.accumulate(x, axis=1)
        m = ((x == r).astype(np.float32) * np.arange(K, dtype=np.float32)[None, :])
        return m, m.max(axis=1, initial=-FMAX).reshape(P, 1)

    from concourse.dve_spec import eq
    _x3 = Src0 + Src1
    spec3 = Spec(body=eq(_x3, scan(AluOp.MAX, _x3)) * Idx, accum=AluOp.MAX,
                 reference=ref3)

    op3 = make("CRF_BT32", spec3, subdim=False)

    # BT32X: same body/accum, but the per-element OUTPUT is redirected to
    # the accumulator chain (block-7 ALU_OUT), so out[k] = running
    # max(eq(x, runmax(x)) * Idx); out[31] = the encoded argmax.  This
    # removes the separate DVE_READ_ACCUMULATOR2 instruction per step.
    from concourse.dve_uop import OutPath, OutSel

    def ref3x(in0, in1, c0, c1, c2):
        P, K = in0.shape
        x = (in0 + in1).astype(np.float32)
        r = np.maximum.accumulate(x, axis=1)
        m = ((x == r).astype(np.float32) * np.arange(K, dtype=np.float32)[None, :])
        acc = np.maximum.accumulate(
            np.maximum(m, -FMAX), axis=1).astype(np.float32)
        return acc, acc[:, -1:].copy()

    spec3x = Spec(body=eq(_x3, scan(AluOp.MAX, _x3)) * Idx, accum=AluOp.MAX,
                  reference=ref3x)

    def patch_out(uops):
        steady = uops[-1]
        assert steady.out_enable[OutPath.WR0_LO]
        steady.out[OutPath.WR0_LO] = OutSel.ALU_OUT

    op3x = make("CRF_BT32X", spec3x, subdim=False, patch=patch_out)
    opF = make_segmax()
    _ops_cache["BT32"] = op3
    _ops_cache["BT32X"] = op3x
    _ops_cache["SEGMAX"] = opF
    return op3


def build_nc(Tn, CH=32):
    if Tn < CH:
        CH = Tn
    op3 = register_custom_ops()
    opF = _ops_cache["SEGMAX"]

    nc = bacc.Bacc("TRN2", target_bir_lowering=False, debug=False,
                   num_devices=NCORES)

    logits = nc.dram_tensor("logits", [PB, Tn, N], F32, kind="ExternalInput")
    trep_d = nc.dram_tensor("trep", [PB, N * N], F32, kind="ExternalInput")
    meq_d = nc.dram_tensor("meq", [PB, Tn], F32, kind="ExternalInput")
    mlt_d = nc.dram_tensor("mlt", [PB, Tn], F32, kind="ExternalInput")
    mrw_d = nc.dram_tensor("mrw", [PB, Tn], I8, kind="ExternalInput")
    mrwf_d = nc.dram_tensor("mrwf", [PB, Tn], F32, kind="ExternalInput")
    mrwc_d = nc.dram_tensor("mrwc", [PB, Tn], F32, kind="ExternalInput")
    irev_d = nc.dram_tensor("irev", [PB, N], F32, kind="ExternalInput")
    trevw_d = nc.dram_tensor("trevw", [PB, N], F32, kind="ExternalInput")
    out_d = nc.dram_tensor("out", [PB, Tn], I32, kind="ExternalOutput")

    SCH = 64  # snapshot-gather chunk (time steps per chunk)
    nsch = (Tn + SCH - 1) // SCH

    with tile.TileContext(nc) as tc:
        with (
            tc.tile_pool(name="consts", bufs=1) as cpool,
            tc.tile_pool(name="states", bufs=1) as spool,
            tc.tile_pool(name="big", bufs=1) as bpool,
            tc.tile_pool(name="lchunks", bufs=2) as lpool,
            tc.tile_pool(name="small", bufs=1) as mpool,
            tc.tile_pool(name="psum", bufs=1, space="PSUM") as ppool,
        ):
            trep = cpool.tile([PB, N * N], F32, tag="trep")
            meq = cpool.tile([PB, Tn], F32, tag="meq")
            mlt = cpool.tile([PB, Tn], F32, tag="mlt")
            mrw = cpool.tile([PB, Tn], I8, tag="mrw")
            mrwf = cpool.tile([PB, Tn], F32, tag="mrwf")
            mrwc = cpool.tile([PB, Tn], F32, tag="mrwc")
            rsc = bpool.tile([PB, 64 * N], F32, tag="rsc")
            irev = cpool.tile([PB, N], F32, tag="irev")
            trevw = cpool.tile([PB, N], F32, tag="trevw")
            states = spool.tile([PB, Tn * N], F32, tag="states")
            scores = bpool.tile([PB, N * N], F32, tag="scores")
            tags = bpool.tile([PB, Tn], F32, tag="tags")
            outi = bpool.tile([PB, Tn], I32, tag="outi")
            scratch = bpool.tile([PB, SCH * N], F32, tag="scratch")
            snap = mpool.tile([PB, N], F32, tag="snap")
            lastt = mpool.tile([PB, 1], F32, tag="lastt")
            snapc = mpool.tile([PB, N], F32, tag="snapc")
            eqs = mpool.tile([PB, N], F32, tag="eqs")
            red = mpool.tile([PB, N], F32, tag="red")
            m1 = mpool.tile([PB, 1], F32, tag="m1")
            onehot = mpool.tile([PB, N], F32, tag="onehot")
            bigoh = mpool.tile([PB, N], F32, tag="bigoh")
            onehotT = mpool.tile([PB, N], F32, tag="onehotT")
            tsel = ppool.tile([PB, N], F32, tag="tsel")

            # first logits chunk + trep gate the first forward step: issue
            # their DMAs before the other constants
            lt0 = lpool.tile([PB, CH * N], F32, tag="lchunk")
            nc.sync.dma_start(
                out=lt0[:].rearrange("p (t v) -> p t v", v=N),
                in_=logits.ap()[:, 0:CH, :],
            )
            nc.sync.dma_start(out=trep[:], in_=trep_d.ap())
            nc.sync.dma_start(out=meq[:], in_=meq_d.ap())
            nc.sync.dma_start(out=mlt[:], in_=mlt_d.ap())
            nc.sync.dma_start(out=mrw[:], in_=mrw_d.ap())
            nc.sync.dma_start(out=mrwf[:], in_=mrwf_d.ap())
            nc.sync.dma_start(out=mrwc[:], in_=mrwc_d.ap())
            nc.sync.dma_start(out=irev[:], in_=irev_d.ap())
            nc.sync.dma_start(out=trevw[:], in_=trevw_d.ap())

            trep3 = trep[:].rearrange("p (c v) -> p c v", v=N)
            scores3 = scores[:].rearrange("p (c v) -> p c v", v=N)

            # ---------------- forward ----------------
            nchunks = (Tn + CH - 1) // CH
            for c in range(nchunks):
                if c == 0:
                    lt = lt0
                else:
                    lt = lpool.tile([PB, CH * N], F32, tag="lchunk")
                    nc.sync.dma_start(
                        out=lt[:].rearrange("p (t v) -> p t v", v=N),
                        in_=logits.ap()[:, c * CH:(c + 1) * CH, :],
                    )
                if c == 0:
                    nc.vector.tensor_copy(out=states[:, 0:N], in_=lt[:, 0:N])
                for i in range(CH):
                    t = c * CH + i
                    if t == 0:
                        continue
                    sprev = states[:, (t - 1) * N: t * N]
                    scur = states[:, t * N: (t + 1) * N]
                    sprev_b = sprev.rearrange("p (o v) -> p o v", o=1) \
                                   .to_broadcast((PB, N, N))
                    # fused add + segmented running-max; segment max lands
                    # in the last column of each row of scores3
                    nc.vector._custom_dve(
                        opF, out=scores3, in0=sprev_b, in1=trep3)
                    nc.vector.tensor_tensor(
                        out=scur.rearrange("p (c o) -> p c o", o=1),
                        in0=scores3[:, :, N - 1:N],
                        in1=lt[:, i * N:(i + 1) * N]
                            .rearrange("p (c o) -> p c o", o=1),
                        op=OP.add)

            # ------------- snapshot gather (masked sum over t) -------------
            # All accumulation on the idle Pool engine (exact: every term
            # but states[len-1] is +/-0.0, and x + 0.0 == x in f32); one
            # final strided reduce on DVE.
            states3 = states[:].rearrange("p (t v) -> p t v", v=N)
            scratch3 = scratch[:].rearrange("p (t v) -> p t v", v=N)
            rsc3s = rsc[:].rearrange("p (t v) -> p t v", v=N)
            nc.gpsimd.memset(scratch[:], 0.0)
            for k in range(nsch):
                t0 = k * SCH
                meqb = meq[:, t0:t0 + SCH] \
                    .rearrange("p (t o) -> p t o", o=1).to_broadcast((PB, SCH, N))
                nc.gpsimd.tensor_tensor(
                    out=rsc3s, in0=states3[:, t0:t0 + SCH, :], in1=meqb,
                    op=OP.mult)
                nc.gpsimd.tensor_tensor(
                    out=scratch3, in0=scratch3, in1=rsc3s, op=OP.add)
            # reduce over t: view scratch as [p, v, t] (strided inner t)
            sc_vt = scratch[:].rearrange("p (t v) -> p v t", v=N)
            nc.vector.tensor_reduce(
                out=snap[:], in_=sc_vt, axis=AX.X, op=OP.add)

            # ------------- last tag (exact first-index argmax) -------------
            nc.vector.tensor_reduce(out=m1[:], in_=snap[:], axis=AX.X, op=OP.max)
            nc.vector.tensor_scalar(
                out=eqs[:], in0=snap[:], scalar1=m1[:], scalar2=None,
                op0=OP.is_equal)
            nc.vector.tensor_tensor(out=red[:], in0=eqs[:], in1=irev[:],
                                    op=OP.mult)
            nc.vector.tensor_reduce(out=lastt[:], in_=red[:],
                                    axis=AX.X, op=OP.max)

            # ------------- rewrite states[t >= len-1] = BIG*onehot ---------
            # Top rows (needed first by the backtrace) via DVE
            # copy_predicated; lower rows rewritten on the idle Pool engine
            # while the backtrace descends (states = states*(1-m) + bigoh*m).
            RS = 896
            RCH = 64
            nc.vector.tensor_scalar(
                out=onehot[:], in0=irev[:], scalar1=lastt[:],
                scalar2=None, op0=OP.is_equal)
            nc.vector.tensor_scalar(
                out=bigoh[:], in0=onehot[:], scalar1=BIG, scalar2=None,
                op0=OP.mult)
            mrwb = mrw[:, RS:].rearrange("p (t o) -> p t o", o=1) \
                .to_broadcast((PB, Tn - RS, N))
            bigohb = bigoh[:].rearrange("p (o v) -> p o v", o=1) \
                .to_broadcast((PB, Tn - RS, N))
            nc.vector.copy_predicated(out=states3[:, RS:, :], mask=mrwb,
                                      data=bigohb)
            rsc3 = rsc[:].rearrange("p (t v) -> p t v", v=N)
            bigohc = bigoh[:].rearrange("p (o v) -> p o v", o=1) \
                .to_broadcast((PB, RCH, N))
            for t0 in range(RS - RCH, -1, -RCH):
                stc = states3[:, t0:t0 + RCH, :]
                mc = mrwc[:, t0:t0 + RCH] \
                    .rearrange("p (t o) -> p t o", o=1) \
                    .to_broadcast((PB, RCH, N))
                mf = mrwf[:, t0:t0 + RCH] \
                    .rearrange("p (t o) -> p t o", o=1) \
                    .to_broadcast((PB, RCH, N))
                nc.gpsimd.tensor_tensor(out=stc, in0=stc, in1=mc, op=OP.mult)
                nc.gpsimd.tensor_tensor(out=rsc3, in0=bigohc, in1=mf,
                                        op=OP.mult)
                nc.gpsimd.tensor_tensor(out=stc, in0=stc, in1=rsc3,
                                        op=OP.add)

            # ---------------- backtrace ----------------
            nc.vector.tensor_copy(out=tags[:, Tn - 1:Tn], in_=lastt[:])
            for t in range(Tn - 2, -1, -1):
                st = states[:, t * N: (t + 1) * N]
                nc.vector.tensor_scalar(
                    out=onehot[:], in0=irev[:], scalar1=tags[:, t + 1:t + 2],
                    scalar2=None, op0=OP.is_equal)
                nc.vector.transpose(out=onehotT[:], in_=onehot[:])
                for blk in range(4):
                    nc.tensor.matmul(
                        out=tsel[blk * N:(blk + 1) * N, :],
                        lhsT=onehotT[blk * N:(blk + 1) * N, :],
                        rhs=trevw[blk * N:(blk + 1) * N, :],
                        start=True, stop=True,
                        tile_position=(blk * N, blk * N))
                nc.vector._custom_dve(
                    op3, out=red[:], in0=tsel[:], in1=st[:, ::-1],
                    accum_out=tags[:, t:t + 1])

            # ---------------- decode + mask + output ----------------
            nc.vector.tensor_scalar(
                out=tags[:], in0=tags[:], scalar1=-1.0, scalar2=31.0,
                op0=OP.mult, op1=OP.add)
            nc.vector.tensor_tensor(out=outi[:], in0=tags[:], in1=mlt[:],
                                    op=OP.mult)
            nc.sync.dma_start(out=out_d.ap(), in_=outi[:])

    nc.compile()
    return nc


def make_inputs_for_core(logits_shard, lens_shard, Tn, Tmat):
    trep = np.ascontiguousarray(Tmat.T).reshape(1, N * N)
    tcol = np.arange(Tn)[None, :]
    lens = lens_shard.astype(np.int64)[:, None]
    meq = (lens == (tcol + 1)).astype(np.float32)
    mlt = (tcol < lens).astype(np.float32)
    mrw = (tcol >= (lens - 1)).astype(np.int8)
    irev = (31.0 - np.arange(N, dtype=np.float32))[None, :]
    rep = lambda a: np.ascontiguousarray(
        np.broadcast_to(a, (PB, a.shape[1])), dtype=np.float32)
    return {
        "logits": np.ascontiguousarray(logits_shard, dtype=np.float32),
        "trep": rep(trep),
        "meq": np.ascontiguousarray(meq, dtype=np.float32),
        "mlt": np.ascontiguousarray(mlt, dtype=np.float32),
        "mrw": np.ascontiguousarray(mrw, dtype=np.int8),
        "mrwf": np.ascontiguousarray(mrw, dtype=np.float32),
        "mrwc": np.ascontiguousarray(1 - mrw, dtype=np.float32),
        "irev": rep(irev),
        "trevw": np.ascontiguousarray(
            np.tile(Tmat[::-1, :].T, (4, 1)), dtype=np.float32),
    }


def last_exec_time_ns():
    return _last_exec_ns[0]


def kernel(logits, transitions, sequence_lengths, _trace=False):
    logits = np.asarray(logits, dtype=np.float32)
    Tmat = np.asarray(transitions, dtype=np.float32)
    lens = np.asarray(sequence_lengths)
    Bn, Tn, Nn = logits.shape
    assert Nn == N and Bn % NCORES == 0

    if Tn not in _nc_cache:
        _nc_cache[Tn] = build_nc(Tn)
    nc = _nc_cache[Tn]

    in_maps = []
    for i in range(NCORES):
        sl = slice(i * PB, (i + 1) * PB)
        in_maps.append(make_inputs_for_core(logits[sl], lens[sl], Tn, Tmat))

    kw = {}
    if _trace:
        kw = dict(trace=True, trace_cores=[0])
    res = run_bass_kernel_spmd(nc, in_maps, core_ids=list(range(NCORES)), **kw)
    _last_exec_ns[0] = getattr(res, "exec_time_ns", None)

    out = np.concatenate([res.results[i]["out"] for i in range(NCORES)], axis=0)
    return out.astype(np.int32)


# revision 8
# speedup vs baseline: 1.0017x; 1.0011x over previous
"""CRF Viterbi decode (B=1024, T=1024, N=32) on 8 TRN2 NeuronCores — v2.

Data-parallel: batch split 128/core, [32,32] transition replicated.

vs v1: (a) snapshot removed from the forward serial chain (batched
masked-sum gather after the forward); (b) forward add+reduce split
across DVE and GPSIMD by cur-segment so each engine chains its own
add->reduce in program order; (c) backtrace freeze handled by a one-time
rewrite states[t] := BIG*onehot(last_tag) for t >= len-1, so the
per-step chain is just onehot -> transpose -> 4x PE matmul -> custom
argmax writing tags[:, t] directly.  All f32 ops identical in value to
the reference (max/adds in same positions), so output stays exact.
"""
import sys
sys.path.insert(0, "/opt/trn_rl_repo")

import numpy as np

import concourse.bass as bass
import concourse.bacc as bacc
import concourse.mybir as mybir
import concourse.tile as tile
from concourse.bass_utils import run_bass_kernel_spmd

F32 = mybir.dt.float32
I32 = mybir.dt.int32
I8 = mybir.dt.int8
AX = mybir.AxisListType
OP = mybir.AluOpType

B, T, N = 1024, 1024, 32
PB = 128
NCORES = 8
BIG = 1.0e6

_ops_cache = {}
_nc_cache = {}
_last_exec_ns = [None]


def register_custom_ops():
    if _ops_cache:
        return _ops_cache["BT32"]
    from concourse.dve_spec import (
        Spec, Src0, Src1, AluOp, lower, Idx, scan, Scan, MaxNeg,
    )
    from concourse.dve_ops import DveOp, OPS, has_src1
    from concourse.dve_uop import DveOpSpec, AluInp
    import concourse.dve_ops as dom

    def make(name, spec, subdim, patch=None):
        for o in OPS:
            if o.name == name:
                return o
        OPS_len = len(OPS)
        dom._SUB_OPCODE_FOR_NAME[name] = dom._CUSTOM_DVE_ROW_BASE + OPS_len
        assert dom._SUB_OPCODE_FOR_NAME[name] < 0x20
        shas = {}
        for ver in ("v3", "v4"):
            uops = lower(spec, ver=ver)
            if patch is not None:
                patch(uops)
            s = DveOpSpec(name=name, opcode=dom.get_dve_sub_opcode(name),
                          uops=uops, rd1_en=has_src1(spec))
            shas[ver] = s.sha(ver)
            dom._COMPILE_CACHE[(name, ver)] = s
        op = DveOp(name, spec, subdim=subdim, uops_sha=shas)
        OPS.append(op)
        dom.CUSTOM_DVE_SPECS[name] = spec
        return op

    def make_segmax():
        """Segmented running-max of (Src0 + Src1) with reset at each subdim
        (row) boundary.  lower() gives the PageIdx FSM skeleton
        [seed, steady(hold), step(combine)]; patch the scan stage so
        steady combines max(reg, incoming) and step resets reg to the
        incoming element (first element of the new segment)."""
        FMAX = np.float32(3.4028235e38)

        def ref(in0, in1, c0, c1, c2):
            x = (in0 + in1).astype(np.float32)
            r = np.maximum.accumulate(x, axis=-1)
            return r, None

        sc = Scan(AluOp.MAX, Src0 + Src1, _subdim_step=MaxNeg)
        spec = Spec(body=sc, reference=ref)

        def patch(uops):
            assert len(uops) == 3, f"expected [seed, steady, step], got {len(uops)}"
            steady, step = uops[1], uops[2]
            dp = steady.datapath_config[1]
            dp.op = AluOp.MAX
            dp.alu_src0 = AluInp.CURR_ALU_OUT
            dp.alu_src1 = AluInp.PREV_ALU_OUT
            dp = step.datapath_config[1]
            dp.op = AluOp.BYPASS
            dp.alu_src0 = AluInp.PREV_ALU_OUT
            dp.alu_src1 = AluInp.PREV_ALU_OUT

        return make("CRF_SEGMAX", spec, subdim=True, patch=patch)

    FMAX = np.float32(3.4028235e38)

    def ref3(in0, in1, c0, c1, c2):
        P, K = in0.shape
        x = (in0 + in1).astype(np.float32)
        r = np.maximum.accumulate(x, axis=1)
        m = ((x == r).astype(np.float32) * np.arange(K, dtype=np.float32)[None, :])
        return m, m.max(axis=1, initial=-FMAX).reshape(P, 1)

    from concourse.dve_spec import eq
    _x3 = Src0 + Src1
    spec3 = Spec(body=eq(_x3, scan(AluOp.MAX, _x3)) * Idx, accum=AluOp.MAX,
                 reference=ref3)

    op3 = make("CRF_BT32", spec3, subdim=False)

    # BT32X: same body/accum, but the per-element OUTPUT is redirected to
    # the accumulator chain (block-7 ALU_OUT), so out[k] = running
    # max(eq(x, runmax(x)) * Idx); out[31] = the encoded argmax.  This
    # removes the separate DVE_READ_ACCUMULATOR2 instruction per step.
    from concourse.dve_uop import OutPath, OutSel

    def ref3x(in0, in1, c0, c1, c2):
        P, K = in0.shape
        x = (in0 + in1).astype(np.float32)
        r = np.maximum.accumulate(x, axis=1)
        m = ((x == r).astype(np.float32) * np.arange(K, dtype=np.float32)[None, :])
        acc = np.maximum.accumulate(
            np.maximum(m, -FMAX), axis=1).astype(np.float32)
        return acc, acc[:, -1:].copy()

    spec3x = Spec(body=eq(_x3, scan(AluOp.MAX, _x3)) * Idx, accum=AluOp.MAX,
                  reference=ref3x)

    def patch_out(uops):
        steady = uops[-1]
        assert steady.out_enable[OutPath.WR0_LO]
        steady.out[OutPath.WR0_LO] = OutSel.ALU_OUT

    op3x = make("CRF_BT32X", spec3x, subdim=False, patch=patch_out)
    opF = make_segmax()
    _ops_cache["BT32"] = op3
    _ops_cache["BT32X"] = op3x
    _ops_cache["SEGMAX"] = opF
    return op3


def build_nc(Tn, CH=16):
    if Tn < CH:
        CH = Tn
    op3 = register_custom_ops()
    opF = _ops_cache["SEGMAX"]

    nc = bacc.Bacc("TRN2", target_bir_lowering=False, debug=False,
                   num_devices=NCORES)

    logits = nc.dram_tensor("logits", [PB, Tn, N], F32, kind="ExternalInput")
    trep_d = nc.dram_tensor("trep", [PB, N * N], F32, kind="ExternalInput")
    meq_d = nc.dram_tensor("meq", [PB, Tn], F32, kind="ExternalInput")
    mlt_d = nc.dram_tensor("mlt", [PB, Tn], F32, kind="ExternalInput")
    mrw_d = nc.dram_tensor("mrw", [PB, Tn], I8, kind="ExternalInput")
    mrwf_d = nc.dram_tensor("mrwf", [PB, Tn], F32, kind="ExternalInput")
    mrwc_d = nc.dram_tensor("mrwc", [PB, Tn], F32, kind="ExternalInput")
    irev_d = nc.dram_tensor("irev", [PB, N], F32, kind="ExternalInput")
    trevw_d = nc.dram_tensor("trevw", [PB, N], F32, kind="ExternalInput")
    out_d = nc.dram_tensor("out", [PB, Tn], I32, kind="ExternalOutput")

    SCH = 64  # snapshot-gather chunk (time steps per chunk)
    nsch = (Tn + SCH - 1) // SCH

    with tile.TileContext(nc) as tc:
        with (
            tc.tile_pool(name="consts", bufs=1) as cpool,
            tc.tile_pool(name="states", bufs=1) as spool,
            tc.tile_pool(name="big", bufs=1) as bpool,
            tc.tile_pool(name="lchunks", bufs=2) as lpool,
            tc.tile_pool(name="small", bufs=1) as mpool,
            tc.tile_pool(name="psum", bufs=1, space="PSUM") as ppool,
        ):
            trep = cpool.tile([PB, N * N], F32, tag="trep")
            meq = cpool.tile([PB, Tn], F32, tag="meq")
            mlt = cpool.tile([PB, Tn], F32, tag="mlt")
            mrw = cpool.tile([PB, Tn], I8, tag="mrw")
            mrwf = cpool.tile([PB, Tn], F32, tag="mrwf")
            mrwc = cpool.tile([PB, Tn], F32, tag="mrwc")
            rsc = bpool.tile([PB, 64 * N], F32, tag="rsc")
            irev = cpool.tile([PB, N], F32, tag="irev")
            trevw = cpool.tile([PB, N], F32, tag="trevw")
            states = spool.tile([PB, Tn * N], F32, tag="states")
            scores = bpool.tile([PB, N * N], F32, tag="scores")
            tags = bpool.tile([PB, Tn], F32, tag="tags")
            outi = bpool.tile([PB, Tn], I32, tag="outi")
            scratch = bpool.tile([PB, SCH * N], F32, tag="scratch")
            snap = mpool.tile([PB, N], F32, tag="snap")
            lastt = mpool.tile([PB, 1], F32, tag="lastt")
            snapc = mpool.tile([PB, N], F32, tag="snapc")
            eqs = mpool.tile([PB, N], F32, tag="eqs")
            red = mpool.tile([PB, N], F32, tag="red")
            m1 = mpool.tile([PB, 1], F32, tag="m1")
            onehot = mpool.tile([PB, N], F32, tag="onehot")
            bigoh = mpool.tile([PB, N], F32, tag="bigoh")
            onehotT = mpool.tile([PB, N], F32, tag="onehotT")
            tsel = ppool.tile([PB, N], F32, tag="tsel")

            # first logits chunk + trep gate the first forward step: issue
            # their DMAs before the other constants
            lt0 = lpool.tile([PB, CH * N], F32, tag="lchunk")
            nc.sync.dma_start(
                out=lt0[:].rearrange("p (t v) -> p t v", v=N),
                in_=logits.ap()[:, 0:CH, :],
            )
            nc.sync.dma_start(out=trep[:], in_=trep_d.ap())
            nc.sync.dma_start(out=meq[:], in_=meq_d.ap())
            nc.sync.dma_start(out=mlt[:], in_=mlt_d.ap())
            nc.sync.dma_start(out=mrw[:], in_=mrw_d.ap())
            nc.sync.dma_start(out=mrwf[:], in_=mrwf_d.ap())
            nc.sync.dma_start(out=mrwc[:], in_=mrwc_d.ap())
            nc.sync.dma_start(out=irev[:], in_=irev_d.ap())
            nc.sync.dma_start(out=trevw[:], in_=trevw_d.ap())

            trep3 = trep[:].rearrange("p (c v) -> p c v", v=N)
            scores3 = scores[:].rearrange("p (c v) -> p c v", v=N)

            # ---------------- forward ----------------
            nchunks = (Tn + CH - 1) // CH
            for c in range(nchunks):
                if c == 0:
                    lt = lt0
                else:
                    lt = lpool.tile([PB, CH * N], F32, tag="lchunk")
                    nc.sync.dma_start(
                        out=lt[:].rearrange("p (t v) -> p t v", v=N),
                        in_=logits.ap()[:, c * CH:(c + 1) * CH, :],
                    )
                if c == 0:
                    nc.vector.tensor_copy(out=states[:, 0:N], in_=lt[:, 0:N])
                for i in range(CH):
                    t = c * CH + i
                    if t == 0:
                        continue
                    sprev = states[:, (t - 1) * N: t * N]
                    scur = states[:, t * N: (t + 1) * N]
                    sprev_b = sprev.rearrange("p (o v) -> p o v", o=1) \
                                   .to_broadcast((PB, N, N))
                    # fused add + segmented running-max; segment max lands
                    # in the last column of each row of scores3
                    nc.vector._custom_dve(
                        opF, out=scores3, in0=sprev_b, in1=trep3)
                    nc.vector.tensor_tensor(
                        out=scur.rearrange("p (c o) -> p c o", o=1),
                        in0=scores3[:, :, N - 1:N],
                        in1=lt[:, i * N:(i + 1) * N]
                            .rearrange("p (c o) -> p c o", o=1),
                        op=OP.add)

            # ------------- snapshot gather (masked sum over t) -------------
            # All accumulation on the idle Pool engine (exact: every term
            # but states[len-1] is +/-0.0, and x + 0.0 == x in f32); one
            # final strided reduce on DVE.
            states3 = states[:].rearrange("p (t v) -> p t v", v=N)
            scratch3 = scratch[:].rearrange("p (t v) -> p t v", v=N)
            rsc3s = rsc[:].rearrange("p (t v) -> p t v", v=N)
            nc.gpsimd.memset(scratch[:], 0.0)
            for k in range(nsch):
                t0 = k * SCH
                meqb = meq[:, t0:t0 + SCH] \
                    .rearrange("p (t o) -> p t o", o=1).to_broadcast((PB, SCH, N))
                nc.gpsimd.tensor_tensor(
                    out=rsc3s, in0=states3[:, t0:t0 + SCH, :], in1=meqb,
                    op=OP.mult)
                nc.gpsimd.tensor_tensor(
                    out=scratch3, in0=scratch3, in1=rsc3s, op=OP.add)
            # reduce over t: view scratch as [p, v, t] (strided inner t)
            sc_vt = scratch[:].rearrange("p (t v) -> p v t", v=N)
            nc.vector.tensor_reduce(
                out=snap[:], in_=sc_vt, axis=AX.X, op=OP.add)

            # ------------- last tag (exact first-index argmax) -------------
            nc.vector.tensor_reduce(out=m1[:], in_=snap[:], axis=AX.X, op=OP.max)
            nc.vector.tensor_scalar(
                out=eqs[:], in0=snap[:], scalar1=m1[:], scalar2=None,
                op0=OP.is_equal)
            nc.vector.tensor_tensor(out=red[:], in0=eqs[:], in1=irev[:],
                                    op=OP.mult)
            nc.vector.tensor_reduce(out=lastt[:], in_=red[:],
                                    axis=AX.X, op=OP.max)

            # ------------- rewrite states[t >= len-1] = BIG*onehot ---------
            # Top rows (needed first by the backtrace) via DVE
            # copy_predicated; lower rows rewritten on the idle Pool engine
            # while the backtrace descends (states = states*(1-m) + bigoh*m).
            RS = 960
            RCH = 64
            nc.vector.tensor_scalar(
                out=onehot[:], in0=irev[:], scalar1=lastt[:],
                scalar2=None, op0=OP.is_equal)
            nc.vector.tensor_scalar(
                out=bigoh[:], in0=onehot[:], scalar1=BIG, scalar2=None,
                op0=OP.mult)
            mrwb = mrw[:, RS:].rearrange("p (t o) -> p t o", o=1) \
                .to_broadcast((PB, Tn - RS, N))
            bigohb = bigoh[:].rearrange("p (o v) -> p o v", o=1) \
                .to_broadcast((PB, Tn - RS, N))
            nc.vector.copy_predicated(out=states3[:, RS:, :], mask=mrwb,
                                      data=bigohb)
            rsc3 = rsc[:].rearrange("p (t v) -> p t v", v=N)
            bigohc = bigoh[:].rearrange("p (o v) -> p o v", o=1) \
                .to_broadcast((PB, RCH, N))
            for t0 in range(RS - RCH, -1, -RCH):
                stc = states3[:, t0:t0 + RCH, :]
                mc = mrwc[:, t0:t0 + RCH] \
                    .rearrange("p (t o) -> p t o", o=1) \
                    .to_broadcast((PB, RCH, N))
                mf = mrwf[:, t0:t0 + RCH] \
                    .rearrange("p (t o) -> p t o", o=1) \
                    .to_broadcast((PB, RCH, N))
                nc.gpsimd.tensor_tensor(out=stc, in0=stc, in1=mc, op=OP.mult)
                nc.gpsimd.tensor_tensor(out=rsc3, in0=bigohc, in1=mf,
                                        op=OP.mult)
                nc.gpsimd.tensor_tensor(out=stc, in0=stc, in1=rsc3,
                                        op=OP.add)

            # ---------------- backtrace ----------------
            nc.vector.tensor_copy(out=tags[:, Tn - 1:Tn], in_=lastt[:])
            for t in range(Tn - 2, -1, -1):
                st = states[:, t * N: (t + 1) * N]
                nc.vector.tensor_scalar(
                    out=onehot[:], in0=irev[:], scalar1=tags[:, t + 1:t + 2],
                    scalar2=None, op0=OP.is_equal)
                nc.vector.transpose(out=onehotT[:], in_=onehot[:])
                for blk in range(4):
                    nc.tensor.matmul(
                        out=tsel[blk * N:(blk + 1) * N, :],
                        lhsT=onehotT[blk * N:(blk + 1) * N, :],
                        rhs=trevw[blk * N:(blk + 1) * N, :],
                        start=True, stop=True,
                        tile_position=(blk * N, blk * N))
                nc.vector._custom_dve(
                    op3, out=red[:], in0=tsel[:], in1=st[:, ::-1],
                    accum_out=tags[:, t:t + 1])

            # ---------------- decode + mask + output ----------------
            nc.vector.tensor_scalar(
                out=tags[:], in0=tags[:], scalar1=-1.0, scalar2=31.0,
                op0=OP.mult, op1=OP.add)
            nc.vector.tensor_tensor(out=outi[:], in0=tags[:], in1=mlt[:],
                                    op=OP.mult)
            nc.sync.dma_start(out=out_d.ap(), in_=outi[:])

    nc.compile()
    return nc


def make_inputs_for_core(logits_shard, lens_shard, Tn, Tmat):
    trep = np.ascontiguousarray(Tmat.T).reshape(1, N * N)
    tcol = np.arange(Tn)[None, :]
    lens = lens_shard.astype(np.int64)[:, None]
    meq = (lens == (tcol + 1)).astype(np.float32)
    mlt = (tcol < lens).astype(np.float32)
    mrw = (tcol >= (lens - 1)).astype(np.int8)
    irev = (31.0 - np.arange(N, dtype=np.float32))[None, :]
    rep = lambda a: np.ascontiguousarray(
        np.broadcast_to(a, (PB, a.shape[1])), dtype=np.float32)
    return {
        "logits": np.ascontiguousarray(logits_shard, dtype=np.float32),
        "trep": rep(trep),
        "meq": np.ascontiguousarray(meq, dtype=np.float32),
        "mlt": np.ascontiguousarray(mlt, dtype=np.float32),
        "mrw": np.ascontiguousarray(mrw, dtype=np.int8),
        "mrwf": np.ascontiguousarray(mrw, dtype=np.float32),
        "mrwc": np.ascontiguousarray(1 - mrw, dtype=np.float32),
        "irev": rep(irev),
        "trevw": np.ascontiguousarray(
            np.tile(Tmat[::-1, :].T, (4, 1)), dtype=np.float32),
    }


def last_exec_time_ns():
    return _last_exec_ns[0]


def kernel(logits, transitions, sequence_lengths, _trace=False):
    logits = np.asarray(logits, dtype=np.float32)
    Tmat = np.asarray(transitions, dtype=np.float32)
    lens = np.asarray(sequence_lengths)
    Bn, Tn, Nn = logits.shape
    assert Nn == N and Bn % NCORES == 0

    if Tn not in _nc_cache:
        _nc_cache[Tn] = build_nc(Tn)
    nc = _nc_cache[Tn]

    in_maps = []
    for i in range(NCORES):
        sl = slice(i * PB, (i + 1) * PB)
        in_maps.append(make_inputs_for_core(logits[sl], lens[sl], Tn, Tmat))

    kw = {}
    if _trace:
        kw = dict(trace=True, trace_cores=[0])
    res = run_bass_kernel_spmd(nc, in_maps, core_ids=list(range(NCORES)), **kw)
    _last_exec_ns[0] = getattr(res, "exec_time_ns", None)

    out = np.concatenate([res.results[i]["out"] for i in range(NCORES)], axis=0)
    return out.astype(np.int32)


# revision 9
# speedup vs baseline: 1.0022x; 1.0004x over previous
"""CRF Viterbi decode (B=1024, T=1024, N=32) on 8 TRN2 NeuronCores — v2.

Data-parallel: batch split 128/core, [32,32] transition replicated.

vs v1: (a) snapshot removed from the forward serial chain (batched
masked-sum gather after the forward); (b) forward add+reduce split
across DVE and GPSIMD by cur-segment so each engine chains its own
add->reduce in program order; (c) backtrace freeze handled by a one-time
rewrite states[t] := BIG*onehot(last_tag) for t >= len-1, so the
per-step chain is just onehot -> transpose -> 4x PE matmul -> custom
argmax writing tags[:, t] directly.  All f32 ops identical in value to
the reference (max/adds in same positions), so output stays exact.
"""
import sys
sys.path.insert(0, "/opt/trn_rl_repo")

import numpy as np

import concourse.bass as bass
import concourse.bacc as bacc
import concourse.mybir as mybir
import concourse.tile as tile
from concourse.bass_utils import run_bass_kernel_spmd

F32 = mybir.dt.float32
I32 = mybir.dt.int32
I8 = mybir.dt.int8
AX = mybir.AxisListType
OP = mybir.AluOpType

B, T, N = 1024, 1024, 32
PB = 128
NCORES = 8
BIG = 1.0e6

_ops_cache = {}
_nc_cache = {}
_last_exec_ns = [None]


def register_custom_ops():
    if _ops_cache:
        return _ops_cache["BT32"]
    from concourse.dve_spec import (
        Spec, Src0, Src1, AluOp, lower, Idx, scan, Scan, MaxNeg,
    )
    from concourse.dve_ops import DveOp, OPS, has_src1
    from concourse.dve_uop import DveOpSpec, AluInp
    import concourse.dve_ops as dom

    def make(name, spec, subdim, patch=None):
        for o in OPS:
            if o.name == name:
                return o
        OPS_len = len(OPS)
        dom._SUB_OPCODE_FOR_NAME[name] = dom._CUSTOM_DVE_ROW_BASE + OPS_len
        assert dom._SUB_OPCODE_FOR_NAME[name] < 0x20
        shas = {}
        for ver in ("v3", "v4"):
            uops = lower(spec, ver=ver)
            if patch is not None:
                patch(uops)
            s = DveOpSpec(name=name, opcode=dom.get_dve_sub_opcode(name),
                          uops=uops, rd1_en=has_src1(spec))
            shas[ver] = s.sha(ver)
            dom._COMPILE_CACHE[(name, ver)] = s
        op = DveOp(name, spec, subdim=subdim, uops_sha=shas)
        OPS.append(op)
        dom.CUSTOM_DVE_SPECS[name] = spec
        return op

    def make_segmax():
        """Segmented running-max of (Src0 + Src1) with reset at each subdim
        (row) boundary.  lower() gives the PageIdx FSM skeleton
        [seed, steady(hold), step(combine)]; patch the scan stage so
        steady combines max(reg, incoming) and step resets reg to the
        incoming element (first element of the new segment)."""
        FMAX = np.float32(3.4028235e38)

        def ref(in0, in1, c0, c1, c2):
            x = (in0 + in1).astype(np.float32)
            r = np.maximum.accumulate(x, axis=-1)
            return r, None

        sc = Scan(AluOp.MAX, Src0 + Src1, _subdim_step=MaxNeg)
        spec = Spec(body=sc, reference=ref)

        def patch(uops):
            assert len(uops) == 3, f"expected [seed, steady, step], got {len(uops)}"
            steady, step = uops[1], uops[2]
            dp = steady.datapath_config[1]
            dp.op = AluOp.MAX
            dp.alu_src0 = AluInp.CURR_ALU_OUT
            dp.alu_src1 = AluInp.PREV_ALU_OUT
            dp = step.datapath_config[1]
            dp.op = AluOp.BYPASS
            dp.alu_src0 = AluInp.PREV_ALU_OUT
            dp.alu_src1 = AluInp.PREV_ALU_OUT

        return make("CRF_SEGMAX", spec, subdim=True, patch=patch)

    FMAX = np.float32(3.4028235e38)

    def ref3(in0, in1, c0, c1, c2):
        P, K = in0.shape
        x = (in0 + in1).astype(np.float32)
        r = np.maximum.accumulate(x, axis=1)
        m = ((x == r).astype(np.float32) * np.arange(K, dtype=np.float32)[None, :])
        return m, m.max(axis=1, initial=-FMAX).reshape(P, 1)

    from concourse.dve_spec import eq
    _x3 = Src0 + Src1
    spec3 = Spec(body=eq(_x3, scan(AluOp.MAX, _x3)) * Idx, accum=AluOp.MAX,
                 reference=ref3)

    op3 = make("CRF_BT32", spec3, subdim=False)

    # BT32X: same body/accum, but the per-element OUTPUT is redirected to
    # the accumulator chain (block-7 ALU_OUT), so out[k] = running
    # max(eq(x, runmax(x)) * Idx); out[31] = the encoded argmax.  This
    # removes the separate DVE_READ_ACCUMULATOR2 instruction per step.
    from concourse.dve_uop import OutPath, OutSel

    def ref3x(in0, in1, c0, c1, c2):
        P, K = in0.shape
        x = (in0 + in1).astype(np.float32)
        r = np.maximum.accumulate(x, axis=1)
        m = ((x == r).astype(np.float32) * np.arange(K, dtype=np.float32)[None, :])
        acc = np.maximum.accumulate(
            np.maximum(m, -FMAX), axis=1).astype(np.float32)
        return acc, acc[:, -1:].copy()

    spec3x = Spec(body=eq(_x3, scan(AluOp.MAX, _x3)) * Idx, accum=AluOp.MAX,
                  reference=ref3x)

    def patch_out(uops):
        steady = uops[-1]
        assert steady.out_enable[OutPath.WR0_LO]
        steady.out[OutPath.WR0_LO] = OutSel.ALU_OUT

    op3x = make("CRF_BT32X", spec3x, subdim=False, patch=patch_out)
    opF = make_segmax()
    _ops_cache["BT32"] = op3
    _ops_cache["BT32X"] = op3x
    _ops_cache["SEGMAX"] = opF
    return op3


def build_nc(Tn, CH=16):
    if Tn < CH:
        CH = Tn
    op3 = register_custom_ops()
    opF = _ops_cache["SEGMAX"]

    nc = bacc.Bacc("TRN2", target_bir_lowering=False, debug=False,
                   num_devices=NCORES)

    logits = nc.dram_tensor("logits", [PB, Tn, N], F32, kind="ExternalInput")
    trep_d = nc.dram_tensor("trep", [PB, N * N], F32, kind="ExternalInput")
    meq_d = nc.dram_tensor("meq", [PB, Tn], F32, kind="ExternalInput")
    mlt_d = nc.dram_tensor("mlt", [PB, Tn], F32, kind="ExternalInput")
    mrw_d = nc.dram_tensor("mrw", [PB, Tn], I8, kind="ExternalInput")
    mrwf_d = nc.dram_tensor("mrwf", [PB, Tn], F32, kind="ExternalInput")
    mrwc_d = nc.dram_tensor("mrwc", [PB, Tn], F32, kind="ExternalInput")
    irev_d = nc.dram_tensor("irev", [PB, N], F32, kind="ExternalInput")
    trevw_d = nc.dram_tensor("trevw", [PB, N], F32, kind="ExternalInput")
    out_d = nc.dram_tensor("out", [PB, Tn], I32, kind="ExternalOutput")

    SCH = 64  # snapshot-gather chunk (time steps per chunk)
    nsch = (Tn + SCH - 1) // SCH

    with tile.TileContext(nc) as tc:
        with (
            tc.tile_pool(name="consts", bufs=1) as cpool,
            tc.tile_pool(name="states", bufs=1) as spool,
            tc.tile_pool(name="big", bufs=1) as bpool,
            tc.tile_pool(name="lchunks", bufs=2) as lpool,
            tc.tile_pool(name="small", bufs=1) as mpool,
            tc.tile_pool(name="psum", bufs=1, space="PSUM") as ppool,
        ):
            trep = cpool.tile([PB, N * N], F32, tag="trep")
            meq = cpool.tile([PB, Tn], F32, tag="meq")
            mlt = cpool.tile([PB, Tn], F32, tag="mlt")
            mrw = cpool.tile([PB, Tn], I8, tag="mrw")
            mrwf = cpool.tile([PB, Tn], F32, tag="mrwf")
            mrwc = cpool.tile([PB, Tn], F32, tag="mrwc")
            rsc = bpool.tile([PB, 64 * N], F32, tag="rsc")
            irev = cpool.tile([PB, N], F32, tag="irev")
            trevw = cpool.tile([PB, N], F32, tag="trevw")
            states = spool.tile([PB, Tn * N], F32, tag="states")
            scores = bpool.tile([PB, N * N], F32, tag="scores")
            tags = bpool.tile([PB, Tn], F32, tag="tags")
            outi = bpool.tile([PB, Tn], I32, tag="outi")
            scratch = bpool.tile([PB, SCH * N], F32, tag="scratch")
            snap = mpool.tile([PB, N], F32, tag="snap")
            lastt = mpool.tile([PB, 1], F32, tag="lastt")
            snapc = mpool.tile([PB, N], F32, tag="snapc")
            eqs = mpool.tile([PB, N], F32, tag="eqs")
            red = mpool.tile([PB, N], F32, tag="red")
            m1 = mpool.tile([PB, 1], F32, tag="m1")
            onehot = mpool.tile([PB, N], F32, tag="onehot")
            bigoh = mpool.tile([PB, N], F32, tag="bigoh")
            onehotT = mpool.tile([PB, N], F32, tag="onehotT")
            tsel = ppool.tile([PB, N], F32, tag="tsel")

            # first logits chunk + trep gate the first forward step: issue
            # their DMAs before the other constants
            lt0 = lpool.tile([PB, CH * N], F32, tag="lchunk")
            nc.sync.dma_start(
                out=lt0[:].rearrange("p (t v) -> p t v", v=N),
                in_=logits.ap()[:, 0:CH, :],
            )
            nc.sync.dma_start(out=trep[:], in_=trep_d.ap())
            nc.sync.dma_start(out=meq[:], in_=meq_d.ap())
            nc.sync.dma_start(out=mlt[:], in_=mlt_d.ap())
            nc.sync.dma_start(out=mrw[:], in_=mrw_d.ap())
            nc.sync.dma_start(out=mrwf[:], in_=mrwf_d.ap())
            nc.sync.dma_start(out=mrwc[:], in_=mrwc_d.ap())
            nc.sync.dma_start(out=irev[:], in_=irev_d.ap())
            nc.sync.dma_start(out=trevw[:], in_=trevw_d.ap())

            trep3 = trep[:].rearrange("p (c v) -> p c v", v=N)
            scores3 = scores[:].rearrange("p (c v) -> p c v", v=N)

            # ---------------- forward ----------------
            nchunks = (Tn + CH - 1) // CH
            for c in range(nchunks):
                if c == 0:
                    lt = lt0
                else:
                    lt = lpool.tile([PB, CH * N], F32, tag="lchunk")
                    nc.sync.dma_start(
                        out=lt[:].rearrange("p (t v) -> p t v", v=N),
                        in_=logits.ap()[:, c * CH:(c + 1) * CH, :],
                    )
                if c == 0:
                    nc.vector.tensor_copy(out=states[:, 0:N], in_=lt[:, 0:N])
                for i in range(CH):
                    t = c * CH + i
                    if t == 0:
                        continue
                    sprev = states[:, (t - 1) * N: t * N]
                    scur = states[:, t * N: (t + 1) * N]
                    sprev_b = sprev.rearrange("p (o v) -> p o v", o=1) \
                                   .to_broadcast((PB, N, N))
                    # fused add + segmented running-max; segment max lands
                    # in the last column of each row of scores3
                    nc.vector._custom_dve(
                        opF, out=scores3, in0=sprev_b, in1=trep3)
                    nc.vector.tensor_tensor(
                        out=scur.rearrange("p (c o) -> p c o", o=1),
                        in0=scores3[:, :, N - 1:N],
                        in1=lt[:, i * N:(i + 1) * N]
                            .rearrange("p (c o) -> p c o", o=1),
                        op=OP.add)

            # ------------- snapshot gather (masked sum over t) -------------
            # All accumulation on the idle Pool engine (exact: every term
            # but states[len-1] is +/-0.0, and x + 0.0 == x in f32); one
            # final strided reduce on DVE.
            states3 = states[:].rearrange("p (t v) -> p t v", v=N)
            scratch3 = scratch[:].rearrange("p (t v) -> p t v", v=N)
            rsc3s = rsc[:].rearrange("p (t v) -> p t v", v=N)
            nc.gpsimd.memset(scratch[:], 0.0)
            for k in range(nsch):
                t0 = k * SCH
                meqb = meq[:, t0:t0 + SCH] \
                    .rearrange("p (t o) -> p t o", o=1).to_broadcast((PB, SCH, N))
                nc.gpsimd.tensor_tensor(
                    out=rsc3s, in0=states3[:, t0:t0 + SCH, :], in1=meqb,
                    op=OP.mult)
                nc.gpsimd.tensor_tensor(
                    out=scratch3, in0=scratch3, in1=rsc3s, op=OP.add)
            # reduce over t: view scratch as [p, v, t] (strided inner t)
            sc_vt = scratch[:].rearrange("p (t v) -> p v t", v=N)
            nc.vector.tensor_reduce(
                out=snap[:], in_=sc_vt, axis=AX.X, op=OP.add)

            # ------------- last tag (exact first-index argmax) -------------
            nc.vector.tensor_reduce(out=m1[:], in_=snap[:], axis=AX.X, op=OP.max)
            nc.vector.tensor_scalar(
                out=eqs[:], in0=snap[:], scalar1=m1[:], scalar2=None,
                op0=OP.is_equal)
            nc.vector.tensor_tensor(out=red[:], in0=eqs[:], in1=irev[:],
                                    op=OP.mult)
            nc.vector.tensor_reduce(out=lastt[:], in_=red[:],
                                    axis=AX.X, op=OP.max)

            # ------------- rewrite states[t >= len-1] = BIG*onehot ---------
            # Top rows (needed first by the backtrace) via DVE
            # copy_predicated; lower rows rewritten on the idle Pool engine
            # while the backtrace descends (states = states*(1-m) + bigoh*m).
            RS = 992
            RCH = 64
            nc.vector.tensor_scalar(
                out=onehot[:], in0=irev[:], scalar1=lastt[:],
                scalar2=None, op0=OP.is_equal)
            nc.vector.tensor_scalar(
                out=bigoh[:], in0=onehot[:], scalar1=BIG, scalar2=None,
                op0=OP.mult)
            mrwb = mrw[:, RS:].rearrange("p (t o) -> p t o", o=1) \
                .to_broadcast((PB, Tn - RS, N))
            bigohb = bigoh[:].rearrange("p (o v) -> p o v", o=1) \
                .to_broadcast((PB, Tn - RS, N))
            nc.vector.copy_predicated(out=states3[:, RS:, :], mask=mrwb,
                                      data=bigohb)
            rsc3 = rsc[:].rearrange("p (t v) -> p t v", v=N)
            bigohc = bigoh[:].rearrange("p (o v) -> p o v", o=1) \
                .to_broadcast((PB, RCH, N))
            for t0 in range(RS - RCH, -1, -RCH):
                stc = states3[:, t0:t0 + RCH, :]
                mc = mrwc[:, t0:t0 + RCH] \
                    .rearrange("p (t o) -> p t o", o=1) \
                    .to_broadcast((PB, RCH, N))
                mf = mrwf[:, t0:t0 + RCH] \
                    .rearrange("p (t o) -> p t o", o=1) \
                    .to_broadcast((PB, RCH, N))
                nc.gpsimd.tensor_tensor(out=stc, in0=stc, in1=mc, op=OP.mult)
                nc.gpsimd.tensor_tensor(out=rsc3, in0=bigohc, in1=mf,
                                        op=OP.mult)
                nc.gpsimd.tensor_tensor(out=stc, in0=stc, in1=rsc3,
                                        op=OP.add)

            # ---------------- backtrace ----------------
            nc.vector.tensor_copy(out=tags[:, Tn - 1:Tn], in_=lastt[:])
            for t in range(Tn - 2, -1, -1):
                st = states[:, t * N: (t + 1) * N]
                nc.vector.tensor_scalar(
                    out=onehot[:], in0=irev[:], scalar1=tags[:, t + 1:t + 2],
                    scalar2=None, op0=OP.is_equal)
                nc.vector.transpose(out=onehotT[:], in_=onehot[:])
                for blk in range(4):
                    nc.tensor.matmul(
                        out=tsel[blk * N:(blk + 1) * N, :],
                        lhsT=onehotT[blk * N:(blk + 1) * N, :],
                        rhs=trevw[blk * N:(blk + 1) * N, :],
                        start=True, stop=True,
                        tile_position=(blk * N, blk * N))
                nc.vector._custom_dve(
                    op3, out=red[:], in0=tsel[:], in1=st[:, ::-1],
                    accum_out=tags[:, t:t + 1])

            # ---------------- decode + mask + output ----------------
            nc.vector.tensor_scalar(
                out=tags[:], in0=tags[:], scalar1=-1.0, scalar2=31.0,
                op0=OP.mult, op1=OP.add)
            nc.vector.tensor_tensor(out=outi[:], in0=tags[:], in1=mlt[:],
                                    op=OP.mult)
            nc.sync.dma_start(out=out_d.ap(), in_=outi[:])

    nc.compile()
    return nc


def make_inputs_for_core(logits_shard, lens_shard, Tn, Tmat):
    trep = np.ascontiguousarray(Tmat.T).reshape(1, N * N)
    tcol = np.arange(Tn)[None, :]
    lens = lens_shard.astype(np.int64)[:, None]
    meq = (lens == (tcol + 1)).astype(np.float32)
    mlt = (tcol < lens).astype(np.float32)
    mrw = (tcol >= (lens - 1)).astype(np.int8)
    irev = (31.0 - np.arange(N, dtype=np.float32))[None, :]
    rep = lambda a: np.ascontiguousarray(
        np.broadcast_to(a, (PB, a.shape[1])), dtype=np.float32)
    return {
        "logits": np.ascontiguousarray(logits_shard, dtype=np.float32),
        "trep": rep(trep),
        "meq": np.ascontiguousarray(meq, dtype=np.float32),
        "mlt": np.ascontiguousarray(mlt, dtype=np.float32),
        "mrw": np.ascontiguousarray(mrw, dtype=np.int8),
        "mrwf": np.ascontiguousarray(mrw, dtype=np.float32),
        "mrwc": np.ascontiguousarray(1 - mrw, dtype=np.float32),
        "irev": rep(irev),
        "trevw": np.ascontiguousarray(
            np.tile(Tmat[::-1, :].T, (4, 1)), dtype=np.float32),
    }


def last_exec_time_ns():
    return _last_exec_ns[0]


def kernel(logits, transitions, sequence_lengths, _trace=False):
    logits = np.asarray(logits, dtype=np.float32)
    Tmat = np.asarray(transitions, dtype=np.float32)
    lens = np.asarray(sequence_lengths)
    Bn, Tn, Nn = logits.shape
    assert Nn == N and Bn % NCORES == 0

    if Tn not in _nc_cache:
        _nc_cache[Tn] = build_nc(Tn)
    nc = _nc_cache[Tn]

    in_maps = []
    for i in range(NCORES):
        sl = slice(i * PB, (i + 1) * PB)
        in_maps.append(make_inputs_for_core(logits[sl], lens[sl], Tn, Tmat))

    kw = {}
    if _trace:
        kw = dict(trace=True, trace_cores=[0])
    res = run_bass_kernel_spmd(nc, in_maps, core_ids=list(range(NCORES)), **kw)
    _last_exec_ns[0] = getattr(res, "exec_time_ns", None)

    out = np.concatenate([res.results[i]["out"] for i in range(NCORES)], axis=0)
    return out.astype(np.int32)
